# revision 21
# baseline (speedup 1.0000x reference)
"""Trainium2 Bass kernel for nn_MultiHeadAttention_firstlayer.

Math notes (derived from the reference):
  - d_k == 1 and the key projection contributes kk[b,i,h] uniformly to every
    element of softmax row (h,b,i,:).  Softmax is shift-invariant, so the
    attention probabilities are exactly softmax(mask_diag(posi_att[h])) --
    independent of k, q, w_k, b_k AND of the batch index.
  - Therefore: attn[h*B+b] = A[h] = softmax_row(posi_att[h] masked at the
    diagonal), and out[b] = LN(concat_h(A[h] @ vv[b,:,h,:]) @ w_fc.T + b_fc).

Distribution (8 cores, no collectives):
  - Core c computes the full forward for batch b=c (all 16 heads), and
    additionally writes the normalized attention matrices for heads 2c, 2c+1.
  - All per-core variation is pushed into the input data via a head
    permutation (owned heads at slots 7 and 15, where their extra fp32
    A-output work overlaps the fc phase), so one SPMD program serves all
    cores.

Layouts (host pre-transposes so every matmul contracts over partitions; the
diagonal mask value is pre-applied to posiT on the host):
  - posiT[h, k, q] = posi_att[h, q, k], with posiT[h, i, i] = -30   (bf16)
  - vT[dm, tok] = v[c].T                 (bf16)
  - wvT = w_v.T (col-permuted), wfcT = w_fc.T (row-permuted)   (bf16)
  - einsum:  pe[d, q]  = sum_k vv[k, d] * E^T[k, q]   (ones column gives the
    softmax denominator s[q] as psum row 64 for free)
  - fc:      pf[q, dm] = sum_hd outT[hd, q] * wfcT[hd, dm]
"""

import sys

sys.path.insert(0, "/opt/trn_rl_repo")

import numpy as np
import ml_dtypes

import concourse.mybir as mybir
import concourse.tile as tile
from concourse import bacc
from concourse.bass_utils import run_bass_kernel_spmd

BF = ml_dtypes.bfloat16
F32 = mybir.dt.float32
BF16 = mybir.dt.bfloat16
AF = mybir.ActivationFunctionType
ALU = mybir.AluOpType
AX = mybir.AxisListType

H, DV, DM, B, L = 16, 64, 1024, 8, 1024
NCORES = 8
LN_EPS = 1e-5
MASK = -30.0  # exp(-30) ~ 9e-14: negligible vs row sums ~1e3

_NC_CACHE = {}


def _build_nc():
    nc = bacc.Bacc("TRN2", target_bir_lowering=False, debug=False, num_devices=NCORES)

    posiT = nc.dram_tensor("posiT", [H, L, L], BF16, kind="ExternalInput")
    vT = nc.dram_tensor("vT", [DM, L], BF16, kind="ExternalInput")
    wvT = nc.dram_tensor("wvT", [DM, H * DV], BF16, kind="ExternalInput")
    wfcT = nc.dram_tensor("wfcT", [H * DV, DM], BF16, kind="ExternalInput")
    bv = nc.dram_tensor("bv", [128, H * DV], BF16, kind="ExternalInput")
    bfc = nc.dram_tensor("bfc", [128, DM], F32, kind="ExternalInput")
    g_bc = nc.dram_tensor("g_bc", [128, DM], F32, kind="ExternalInput")
    b_bc = nc.dram_tensor("b_bc", [128, DM], F32, kind="ExternalInput")

    A_out = nc.dram_tensor("A_out", [2, L, L], F32, kind="ExternalOutput")
    y_out = nc.dram_tensor("y_out", [L, DM], F32, kind="ExternalOutput")

    with tile.TileContext(nc) as tc:
        with tc.tile_pool(name="const", bufs=1) as cp, \
             tc.tile_pool(name="work", bufs=2) as wp, \
             tc.tile_pool(name="ln", bufs=2) as lp, \
             tc.tile_pool(name="ps", bufs=2, space="PSUM") as ps:
            wfc_sb = cp.tile([128, 8 * DM], BF16, tag="wfcall", name="wfcall")
            vv_sb = [cp.tile([128, H * 65], BF16, tag=f"vv{i}", name=f"vv{i}")
                     for i in range(8)]
            outT_sb = [cp.tile([128, L], BF16, tag=f"oT{i}", name=f"oT{i}")
                       for i in range(8)]

            bv_sb = cp.tile([128, H * DV], BF16, tag="bv")
            bfc_sb = cp.tile([128, DM], F32, tag="bfc")
            g_sb = cp.tile([128, DM], F32, tag="g")
            b_sb = cp.tile([128, DM], F32, tag="b")

            vT_sb = wp.tile([128, 8 * L], BF16, tag="stage", name="vTall",
                            bufs=4)
            wv_sb = wp.tile([128, 8 * H * DV], BF16, tag="stage", name="wvall",
                            bufs=4)
            nc.sync.dma_start(bv_sb[:], bv.ap())
            nc.sync.dma_start(bfc_sb[:], bfc.ap())
            nc.sync.dma_start(g_sb[:], g_bc.ap())
            nc.sync.dma_start(b_sb[:], b_bc.ap())
            for i in range(8):
                nc.sync.dma_start(vT_sb[:, i * L:(i + 1) * L],
                                  vT.ap()[i * 128:(i + 1) * 128, :])
                nc.sync.dma_start(wv_sb[:, i * H * DV:(i + 1) * H * DV],
                                  wvT.ap()[i * 128:(i + 1) * 128, :])
            pre_stage = {}
            for h in range(2):
                st = wp.tile([128, 8 * L], BF16, tag="stage", name=f"st{h}",
                             bufs=4)
                nc.sync.dma_start(
                    st.rearrange("p (kt q) -> p kt q", q=L),
                    posiT.ap()[h].rearrange("(kt p) q -> p kt q", p=128),
                )
                pre_stage[h] = st
            nc.sync.dma_start(wfc_sb.rearrange("p (i q) -> p i q", q=DM),
                              wfcT.ap().rearrange("(i p) q -> p i q", p=128))

            # ---- Phase 1: vv[tok, hd] = v @ w_v.T + b_v, stored slot-major in
            #      65-col blocks per head (64 values + a ones column).
            for tokt in range(8):
                pv = ps.tile([128, H * DV], F32, tag="acc", name=f"pv{tokt}")
                for dmt in range(8):
                    for n in range(2):
                        nc.tensor.matmul(
                            pv[:, n * 512:(n + 1) * 512],
                            vT_sb[:, dmt * L + tokt * 128: dmt * L + (tokt + 1) * 128],
                            wv_sb[:, dmt * H * DV + n * 512: dmt * H * DV + (n + 1) * 512],
                            start=(dmt == 0), stop=(dmt == 7),
                        )
                src = pv.rearrange("p (h d) -> p h d", d=DV)
                dst = vv_sb[tokt].rearrange("p (h c) -> p h c", c=65)
                nc.vector.tensor_tensor(
                    dst[:, :, 0:DV], src,
                    bv_sb.rearrange("p (h d) -> p h d", d=DV), ALU.add)
                nc.vector.memset(dst[:, :, DV], 1.0)

            # ---- Phase 2: per head: E^T = exp(posiT); einsum + denominator;
            #      normalize outT; owned heads (slots 7, 15) emit A^T in fp32,
            #      re-computing exp in fp32 from the preserved posi staging
            #      tile.  Slot 7's A work is flushed in quarters at the top of
            #      heads 9/10 so it never stalls the recip->bcast->norm chain.
            own_pending = []

            def emit_own_quarter(oh, oslot, ostage, obc, qtr):
                aa = wp.tile([128, 2 * L], F32, tag="aa", name=f"aa{oh}_{qtr}")
                for j in range(2):
                    kt = qtr * 2 + j
                    af = wp.tile([128, L], F32, tag="af", name=f"af{oh}_{kt}")
                    nc.scalar.activation(af[:], ostage[:, kt * L:(kt + 1) * L],
                                         AF.Exp)
                    nc.vector.tensor_tensor(
                        aa[:, j * L:(j + 1) * L], af[:], obc[:], ALU.mult)
                nc.gpsimd.dma_start(
                    A_out.ap()[oslot, qtr * 256:(qtr + 1) * 256, :]
                         .rearrange("(kt p) q -> p kt q", p=128),
                    aa.rearrange("p (kt q) -> p kt q", q=L))

            for h in range(H):
                owned = h in (13, 15)
                if h in pre_stage:
                    stage = pre_stage[h]
                else:
                    stage = wp.tile([128, 8 * L], BF16, tag="stage",
                                    name=f"st{h}", bufs=4)
                    nc.sync.dma_start(
                        stage.rearrange("p (kt q) -> p kt q", q=L),
                        posiT.ap()[h].rearrange("(kt p) q -> p kt q", p=128),
                    )
                if owned:
                    E = wp.tile([128, 8 * L], BF16, tag="Eown", name=f"E{h}",
                                bufs=1)
                else:
                    E = stage
                for c2 in range(2):  # exp in 2 chunks of 4096 for pipelining
                    nc.scalar.activation(
                        E[:, c2 * 4096:(c2 + 1) * 4096],
                        stage[:, c2 * 4096:(c2 + 1) * 4096], AF.Exp)
                pe = ps.tile([65, L], F32, tag="pe", name=f"pe{h}")
                for kt in range(8):
                    for n in range(2):
                        nc.tensor.matmul(
                            pe[:, n * 512:(n + 1) * 512],
                            vv_sb[kt][:, h * 65:(h + 1) * 65],
                            E[:, kt * L + n * 512: kt * L + n * 512 + 512],
                            start=(kt == 0), stop=(kt == 7),
                        )
                rc = wp.tile([1, L], F32, tag="rc", name=f"rc{h}")
                nc.vector.reciprocal(rc[:], pe[64:65, :])
                bc = wp.tile([128, L], F32, tag="bcown" if owned else "bc",
                             name=f"bc{h}", bufs=2)
                nc.gpsimd.partition_broadcast(bc[:], rc[:])

                r0 = (h % 2) * 64
                nc.vector.tensor_tensor(
                    outT_sb[h // 2][r0:r0 + 64, :], pe[0:64, :],
                    bc[r0:r0 + 64, :], ALU.mult,
                )
                if owned:
                    own_pending.append((h, stage, bc))
            # both owned slots' A outputs at the tail (overlap the fc matmuls)
            for oslot, (oh, ostage, obc) in enumerate(own_pending):
                for qtr in range(4):
                    emit_own_quarter(oh, oslot, ostage, obc, qtr)

            # ---- Phase 3: fc + bias + LayerNorm (LN chain pipelined one
            #      qt behind the matmuls so DVE never head-of-line blocks).
            def ln_chain(qt, yt):
                sm = lp.tile([128, 1], F32, tag="sm", name=f"sm{qt}")
                nc.vector.reduce_sum(sm[:], yt[:], axis=AX.X)
                dummy = lp.tile([128, DM], F32, tag="dummy", name=f"dm{qt}",
                                bufs=1)
                sq = lp.tile([128, 1], F32, tag="sq", name=f"sq{qt}")
                nc.scalar.activation(dummy[:], yt[:], AF.Square, accum_out=sq[:])
                nmu = lp.tile([128, 1], F32, tag="nmu", name=f"nmu{qt}")
                nc.vector.tensor_scalar_mul(nmu[:], sm[:], -1.0 / DM)
                musq = lp.tile([128, 1], F32, tag="musq", name=f"musq{qt}")
                nc.vector.tensor_tensor(musq[:], nmu[:], nmu[:], ALU.mult)
                var = lp.tile([128, 1], F32, tag="var", name=f"var{qt}")
                nc.vector.tensor_scalar(var[:], sq[:], 1.0 / DM, LN_EPS,
                                        ALU.mult, ALU.add)
                nc.vector.tensor_sub(var[:], var[:], musq[:])
                srt = lp.tile([128, 1], F32, tag="srt", name=f"srt{qt}")
                nc.scalar.activation(srt[:], var[:], AF.Sqrt)
                r = lp.tile([128, 1], F32, tag="r", name=f"r{qt}")
                nc.vector.reciprocal(r[:], srt[:])
                t = lp.tile([128, 1], F32, tag="t", name=f"t{qt}")
                nc.vector.tensor_tensor(t[:], r[:], r[:], ALU.mult)
                nc.vector.tensor_tensor(t[:], t[:], var[:], ALU.mult)
                nc.vector.tensor_scalar(t[:], t[:], -0.5, 1.5, ALU.mult, ALU.add)
                nc.vector.tensor_tensor(r[:], r[:], t[:], ALU.mult)
                nc.vector.tensor_scalar(yt[:], yt[:], nmu[:], r[:],
                                        ALU.add, ALU.mult)
                nc.vector.tensor_tensor(yt[:], yt[:], g_sb[:], ALU.mult)
                nc.vector.tensor_tensor(yt[:], yt[:], b_sb[:], ALU.add)
                nc.gpsimd.dma_start(y_out.ap()[qt * 128:(qt + 1) * 128, :], yt[:])

            pending = None
            for qt in range(8):
                pf = ps.tile([128, DM], F32, tag="acc", name=f"pf{qt}")
                for hdt in range(8):
                    for n in range(2):
                        nc.tensor.matmul(
                            pf[:, n * 512:(n + 1) * 512],
                            outT_sb[hdt][:, qt * 128:(qt + 1) * 128],
                            wfc_sb[:, hdt * DM + n * 512: hdt * DM + (n + 1) * 512],
                            start=(hdt == 0), stop=(hdt == 7),
                        )
                yt = lp.tile([128, DM], F32, tag="yt", name=f"yt{qt}", bufs=3)
                nc.vector.tensor_tensor(yt[:], pf[:], bfc_sb[:], ALU.add)
                if pending is not None:
                    ln_chain(*pending)
                pending = (qt, yt)
            ln_chain(*pending)

    nc.compile()
    return nc


def _get_nc():
    if "nc" not in _NC_CACHE:
        _NC_CACHE["nc"] = _build_nc()
    return _NC_CACHE["nc"]


def make_in_maps(v, posi_att, w_v, b_v, w_fc, b_fc, ln_g, ln_b):
    v = np.asarray(v, np.float32)
    posi = np.asarray(posi_att, np.float32)
    w_v = np.asarray(w_v, np.float32)
    b_v = np.asarray(b_v, np.float32)
    w_fc = np.asarray(w_fc, np.float32)
    b_fc = np.asarray(b_fc, np.float32)
    ln_g = np.asarray(ln_g, np.float32)
    ln_b = np.asarray(ln_b, np.float32)

    posiT_all = posi.transpose(0, 2, 1).astype(BF)           # [H, k, q]
    ii = np.arange(L)
    posiT_all[:, ii, ii] = np.float32(MASK)                  # diagonal mask
    wvT_full = w_v.T.astype(BF)                              # [DM, H*DV]
    wfcT_full = w_fc.T.astype(BF)                            # [H*DV, DM]
    g_bc = np.ascontiguousarray(np.broadcast_to(ln_g, (128, DM)), np.float32)
    b_bc = np.ascontiguousarray(np.broadcast_to(ln_b, (128, DM)), np.float32)
    bfc_bc = np.ascontiguousarray(np.broadcast_to(b_fc, (128, DM)), np.float32)

    in_maps = []
    for c in range(NCORES):
        others = [h for h in range(H) if h // 2 != c]
        perm = others[:13] + [2 * c] + others[13:] + [2 * c + 1]
        colidx = np.concatenate([np.arange(p * DV, (p + 1) * DV) for p in perm])
        in_maps.append({
            "posiT": np.ascontiguousarray(posiT_all[perm]),
            "vT": np.ascontiguousarray(v[c].T).astype(BF),
            "wvT": np.ascontiguousarray(wvT_full[:, colidx]),
            "wfcT": np.ascontiguousarray(wfcT_full[colidx, :]),
            "bv": np.ascontiguousarray(
                np.broadcast_to(b_v[colidx], (128, H * DV))).astype(BF),
            "bfc": bfc_bc,
            "g_bc": g_bc,
            "b_bc": b_bc,
        })
    return in_maps


def assemble_outputs(results):
    out = np.stack([np.asarray(results[c]["y_out"]) for c in range(NCORES)])
    attn_h = np.empty((H, L, L), np.float32)
    for c in range(NCORES):
        attn_h[2 * c] = results[c]["A_out"][0].T
        attn_h[2 * c + 1] = results[c]["A_out"][1].T
    attn = np.broadcast_to(attn_h[:, None], (H, B, L, L)).reshape(H * B, L, L)
    return out, attn


def kernel(q=None, k=None, v=None, posi_att=None, w_k=None, b_k=None,
           w_v=None, b_v=None, w_fc=None, b_fc=None, ln_g=None, ln_b=None, **_):
    nc = _get_nc()
    in_maps = make_in_maps(v, posi_att, w_v, b_v, w_fc, b_fc, ln_g, ln_b)
    last_err = None
    for _attempt in range(4):
        try:
            res = run_bass_kernel_spmd(nc, in_maps, core_ids=list(range(NCORES)))
            return assemble_outputs(res.results)
        except Exception as e:  # transient backend execution errors
            last_err = e
            import time as _time
            _time.sleep(5.0)
    raise last_err


# revision 25
# speedup vs baseline: 1.2831x; 1.2831x over previous
"""Trainium2 Bass kernel for nn_MultiHeadAttention_firstlayer.

Math notes (derived from the reference):
  - d_k == 1 and the key projection contributes kk[b,i,h] uniformly to every
    element of softmax row (h,b,i,:).  Softmax is shift-invariant, so the
    attention probabilities are exactly softmax(mask_diag(posi_att[h])) --
    independent of k, q, w_k, b_k AND of the batch index.
  - Therefore: attn[h*B+b] = A[h] = softmax_row(posi_att[h] masked at the
    diagonal), and out[b] = LN(concat_h(A[h] @ vv[b,:,h,:]) @ w_fc.T + b_fc).

Distribution (8 cores, no collectives):
  - Core c computes the full forward for batch b=c (all 16 heads), and
    additionally writes the normalized attention matrices for heads 2c, 2c+1.
  - All per-core variation is pushed into the input data via a head
    permutation (owned heads at slots 7 and 15, where their extra fp32
    A-output work overlaps the fc phase), so one SPMD program serves all
    cores.

Layouts (host pre-transposes so every matmul contracts over partitions; the
diagonal mask value is pre-applied to posiT on the host):
  - posiT[h, k, q] = posi_att[h, q, k], with posiT[h, i, i] = -30   (bf16)
  - vT[dm, tok] = v[c].T                 (bf16)
  - wvT = w_v.T (col-permuted), wfcT = w_fc.T (row-permuted)   (bf16)
  - einsum:  pe[d, q]  = sum_k vv[k, d] * E^T[k, q]   (ones column gives the
    softmax denominator s[q] as psum row 64 for free)
  - fc:      pf[q, dm] = sum_hd outT[hd, q] * wfcT[hd, dm]
"""

import sys

sys.path.insert(0, "/opt/trn_rl_repo")

import numpy as np
import ml_dtypes

import concourse.mybir as mybir
import concourse.tile as tile
from concourse import bacc
from concourse.bass_utils import run_bass_kernel_spmd

BF = ml_dtypes.bfloat16
F32 = mybir.dt.float32
BF16 = mybir.dt.bfloat16
AF = mybir.ActivationFunctionType
ALU = mybir.AluOpType
AX = mybir.AxisListType

H, DV, DM, B, L = 16, 64, 1024, 8, 1024
NCORES = 8
LN_EPS = 1e-5
MASK = -30.0  # exp(-30) ~ 9e-14: negligible vs row sums ~1e3

_NC_CACHE = {}


def _build_nc():
    nc = bacc.Bacc("TRN2", target_bir_lowering=False, debug=False, num_devices=NCORES)

    posiT = nc.dram_tensor("posiT", [H, L, L], BF16, kind="ExternalInput")
    vT = nc.dram_tensor("vT", [DM, L], BF16, kind="ExternalInput")
    wvT = nc.dram_tensor("wvT", [DM, H * DV], BF16, kind="ExternalInput")
    wfcT = nc.dram_tensor("wfcT", [H * DV, DM], BF16, kind="ExternalInput")
    bv = nc.dram_tensor("bv", [128, H * DV], BF16, kind="ExternalInput")
    bfc = nc.dram_tensor("bfc", [128, DM], F32, kind="ExternalInput")
    g_bc = nc.dram_tensor("g_bc", [128, DM], F32, kind="ExternalInput")
    b_bc = nc.dram_tensor("b_bc", [128, DM], F32, kind="ExternalInput")

    A_out = nc.dram_tensor("A_out", [2, L, L], F32, kind="ExternalOutput")
    y_out = nc.dram_tensor("y_out", [L, DM], F32, kind="ExternalOutput")

    with tile.TileContext(nc) as tc:
        with tc.tile_pool(name="const", bufs=1) as cp, \
             tc.tile_pool(name="work", bufs=2) as wp, \
             tc.tile_pool(name="ln", bufs=2) as lp, \
             tc.tile_pool(name="ps", bufs=2, space="PSUM") as ps:
            wfc_sb = cp.tile([128, 8 * DM], BF16, tag="wfcall", name="wfcall")
            vv_sb = [cp.tile([128, H * 65], BF16, tag=f"vv{i}", name=f"vv{i}")
                     for i in range(8)]
            outT_sb = [cp.tile([128, L], BF16, tag=f"oT{i}", name=f"oT{i}")
                       for i in range(8)]

            bv_sb = cp.tile([128, H * DV], BF16, tag="bv")
            bfc_sb = cp.tile([128, DM], F32, tag="bfc")
            g_sb = cp.tile([128, DM], F32, tag="g")
            b_sb = cp.tile([128, DM], F32, tag="b")

            vT_sb = wp.tile([128, 8 * L], BF16, tag="stage", name="vTall",
                            bufs=4)
            wv_sb = wp.tile([128, 8 * H * DV], BF16, tag="stage", name="wvall",
                            bufs=4)
            nc.sync.dma_start(bv_sb[:], bv.ap())
            nc.sync.dma_start(bfc_sb[:], bfc.ap())
            nc.sync.dma_start(g_sb[:], g_bc.ap())
            nc.sync.dma_start(b_sb[:], b_bc.ap())
            for i in range(8):
                nc.sync.dma_start(vT_sb[:, i * L:(i + 1) * L],
                                  vT.ap()[i * 128:(i + 1) * 128, :])
                nc.sync.dma_start(wv_sb[:, i * H * DV:(i + 1) * H * DV],
                                  wvT.ap()[i * 128:(i + 1) * 128, :])
            pre_stage = {}
            for h in range(2):
                st = wp.tile([128, 8 * L], BF16, tag="stage", name=f"st{h}",
                             bufs=4)
                nc.sync.dma_start(
                    st.rearrange("p (kt q) -> p kt q", q=L),
                    posiT.ap()[h].rearrange("(kt p) q -> p kt q", p=128),
                )
                pre_stage[h] = st
            nc.sync.dma_start(wfc_sb.rearrange("p (i q) -> p i q", q=DM),
                              wfcT.ap().rearrange("(i p) q -> p i q", p=128))

            # ---- Phase 1: vv[tok, hd] = v @ w_v.T + b_v, stored slot-major in
            #      65-col blocks per head (64 values + a ones column).
            for tokt in range(8):
                pv = ps.tile([128, H * DV], F32, tag="acc", name=f"pv{tokt}")
                for dmt in range(8):
                    for n in range(2):
                        nc.tensor.matmul(
                            pv[:, n * 512:(n + 1) * 512],
                            vT_sb[:, dmt * L + tokt * 128: dmt * L + (tokt + 1) * 128],
                            wv_sb[:, dmt * H * DV + n * 512: dmt * H * DV + (n + 1) * 512],
                            start=(dmt == 0), stop=(dmt == 7),
                        )
                src = pv.rearrange("p (h d) -> p h d", d=DV)
                dst = vv_sb[tokt].rearrange("p (h c) -> p h c", c=65)
                nc.vector.tensor_tensor(
                    dst[:, :, 0:DV], src,
                    bv_sb.rearrange("p (h d) -> p h d", d=DV), ALU.add)
                nc.vector.memset(dst[:, :, DV], 1.0)

            # ---- Phase 2: per head: E^T = exp(posiT); einsum + denominator;
            #      normalize outT; owned heads (slots 7, 15) emit A^T in fp32,
            #      re-computing exp in fp32 from the preserved posi staging
            #      tile.  Slot 7's A work is flushed in quarters at the top of
            #      heads 9/10 so it never stalls the recip->bcast->norm chain.
            own_pending = []

            def emit_own_quarter(oh, oslot, ostage, obc, qtr):
                aa = wp.tile([128, 2 * L], F32, tag="aa", name=f"aa{oh}_{qtr}")
                for j in range(2):
                    kt = qtr * 2 + j
                    af = wp.tile([128, L], F32, tag="af", name=f"af{oh}_{kt}")
                    nc.scalar.activation(af[:], ostage[:, kt * L:(kt + 1) * L],
                                         AF.Exp)
                    nc.vector.tensor_tensor(
                        aa[:, j * L:(j + 1) * L], af[:], obc[:], ALU.mult)
                nc.gpsimd.dma_start(
                    A_out.ap()[oslot, qtr * 256:(qtr + 1) * 256, :]
                         .rearrange("(kt p) q -> p kt q", p=128),
                    aa.rearrange("p (kt q) -> p kt q", q=L))

            for h in range(H):
                owned = h in (13, 15)
                if h in pre_stage:
                    stage = pre_stage[h]
                else:
                    stage = wp.tile([128, 8 * L], BF16, tag="stage",
                                    name=f"st{h}", bufs=4)
                    nc.sync.dma_start(
                        stage.rearrange("p (kt q) -> p kt q", q=L),
                        posiT.ap()[h].rearrange("(kt p) q -> p kt q", p=128),
                    )
                if owned:
                    E = wp.tile([128, 8 * L], BF16, tag="Eown", name=f"E{h}",
                                bufs=1)
                else:
                    E = stage
                for c2 in range(2):  # exp in 2 chunks of 4096 for pipelining
                    nc.scalar.activation(
                        E[:, c2 * 4096:(c2 + 1) * 4096],
                        stage[:, c2 * 4096:(c2 + 1) * 4096], AF.Exp)
                pe = ps.tile([65, L], F32, tag="pe", name=f"pe{h}")
                for kt in range(8):
                    for n in range(2):
                        nc.tensor.matmul(
                            pe[:, n * 512:(n + 1) * 512],
                            vv_sb[kt][:, h * 65:(h + 1) * 65],
                            E[:, kt * L + n * 512: kt * L + n * 512 + 512],
                            start=(kt == 0), stop=(kt == 7),
                        )
                rc = wp.tile([1, L], F32, tag="rc", name=f"rc{h}")
                nc.vector.reciprocal(rc[:], pe[64:65, :])
                bc = wp.tile([128, L], F32, tag="bcown" if owned else "bc",
                             name=f"bc{h}", bufs=2)
                nc.gpsimd.partition_broadcast(bc[:], rc[:])

                r0 = (h % 2) * 64
                nc.vector.tensor_tensor(
                    outT_sb[h // 2][r0:r0 + 64, :], pe[0:64, :],
                    bc[r0:r0 + 64, :], ALU.mult,
                )
                if owned:
                    own_pending.append((h, stage, bc))
            # both owned slots' A outputs at the tail (overlap the fc matmuls)
            for oslot, (oh, ostage, obc) in enumerate(own_pending):
                for qtr in range(4):
                    emit_own_quarter(oh, oslot, ostage, obc, qtr)

            # ---- Phase 3: fc + bias + LayerNorm (LN chain pipelined one
            #      qt behind the matmuls so DVE never head-of-line blocks).
            def ln_chain(qt, yt):
                sm = lp.tile([128, 1], F32, tag="sm", name=f"sm{qt}")
                nc.vector.reduce_sum(sm[:], yt[:], axis=AX.X)
                dummy = lp.tile([128, DM], F32, tag="dummy", name=f"dm{qt}",
                                bufs=1)
                sq = lp.tile([128, 1], F32, tag="sq", name=f"sq{qt}")
                nc.scalar.activation(dummy[:], yt[:], AF.Square, accum_out=sq[:])
                nmu = lp.tile([128, 1], F32, tag="nmu", name=f"nmu{qt}")
                nc.vector.tensor_scalar_mul(nmu[:], sm[:], -1.0 / DM)
                musq = lp.tile([128, 1], F32, tag="musq", name=f"musq{qt}")
                nc.vector.tensor_tensor(musq[:], nmu[:], nmu[:], ALU.mult)
                var = lp.tile([128, 1], F32, tag="var", name=f"var{qt}")
                nc.vector.tensor_scalar(var[:], sq[:], 1.0 / DM, LN_EPS,
                                        ALU.mult, ALU.add)
                nc.vector.tensor_sub(var[:], var[:], musq[:])
                srt = lp.tile([128, 1], F32, tag="srt", name=f"srt{qt}")
                nc.scalar.activation(srt[:], var[:], AF.Sqrt)
                r = lp.tile([128, 1], F32, tag="r", name=f"r{qt}")
                nc.vector.reciprocal(r[:], srt[:])
                t = lp.tile([128, 1], F32, tag="t", name=f"t{qt}")
                nc.vector.tensor_tensor(t[:], r[:], r[:], ALU.mult)
                nc.vector.tensor_tensor(t[:], t[:], var[:], ALU.mult)
                nc.vector.tensor_scalar(t[:], t[:], -0.5, 1.5, ALU.mult, ALU.add)
                nc.vector.tensor_tensor(r[:], r[:], t[:], ALU.mult)
                nc.vector.tensor_scalar(yt[:], yt[:], nmu[:], r[:],
                                        ALU.add, ALU.mult)
                nc.vector.tensor_tensor(yt[:], yt[:], g_sb[:], ALU.mult)
                nc.vector.tensor_tensor(yt[:], yt[:], b_sb[:], ALU.add)
                nc.gpsimd.dma_start(y_out.ap()[qt * 128:(qt + 1) * 128, :], yt[:])

            pending = None
            for qt in range(8):
                pf = ps.tile([128, DM], F32, tag="acc", name=f"pf{qt}")
                for hdt in range(8):
                    for n in range(2):
                        nc.tensor.matmul(
                            pf[:, n * 512:(n + 1) * 512],
                            outT_sb[hdt][:, qt * 128:(qt + 1) * 128],
                            wfc_sb[:, hdt * DM + n * 512: hdt * DM + (n + 1) * 512],
                            start=(hdt == 0), stop=(hdt == 7),
                        )
                yt = lp.tile([128, DM], F32, tag="yt", name=f"yt{qt}", bufs=3)
                nc.vector.tensor_tensor(yt[:], pf[:], bfc_sb[:], ALU.add)
                if pending is not None:
                    ln_chain(*pending)
                pending = (qt, yt)
            ln_chain(*pending)

    nc.compile()
    return nc


def _get_nc():
    if "nc" not in _NC_CACHE:
        _NC_CACHE["nc"] = _build_nc()
    return _NC_CACHE["nc"]


def make_in_maps(v, posi_att, w_v, b_v, w_fc, b_fc, ln_g, ln_b):
    v = np.asarray(v, np.float32)
    posi = np.asarray(posi_att, np.float32)
    w_v = np.asarray(w_v, np.float32)
    b_v = np.asarray(b_v, np.float32)
    w_fc = np.asarray(w_fc, np.float32)
    b_fc = np.asarray(b_fc, np.float32)
    ln_g = np.asarray(ln_g, np.float32)
    ln_b = np.asarray(ln_b, np.float32)

    posiT_all = posi.transpose(0, 2, 1).astype(BF)           # [H, k, q]
    ii = np.arange(L)
    posiT_all[:, ii, ii] = np.float32(MASK)                  # diagonal mask
    wvT_full = w_v.T.astype(BF)                              # [DM, H*DV]
    wfcT_full = w_fc.T.astype(BF)                            # [H*DV, DM]
    g_bc = np.ascontiguousarray(np.broadcast_to(ln_g, (128, DM)), np.float32)
    b_bc = np.ascontiguousarray(np.broadcast_to(ln_b, (128, DM)), np.float32)
    bfc_bc = np.ascontiguousarray(np.broadcast_to(b_fc, (128, DM)), np.float32)

    in_maps = []
    for c in range(NCORES):
        others = [h for h in range(H) if h // 2 != c]
        perm = others[:13] + [2 * c] + others[13:] + [2 * c + 1]
        colidx = np.concatenate([np.arange(p * DV, (p + 1) * DV) for p in perm])
        in_maps.append({
            "posiT": np.ascontiguousarray(posiT_all[perm]),
            "vT": np.ascontiguousarray(v[c].T).astype(BF),
            "wvT": np.ascontiguousarray(wvT_full[:, colidx]),
            "wfcT": np.ascontiguousarray(wfcT_full[colidx, :]),
            "bv": np.ascontiguousarray(
                np.broadcast_to(b_v[colidx], (128, H * DV))).astype(BF),
            "bfc": bfc_bc,
            "g_bc": g_bc,
            "b_bc": b_bc,
        })
    return in_maps


def assemble_outputs(results):
    out = np.stack([np.asarray(results[c]["y_out"]) for c in range(NCORES)])
    attn_h = np.empty((H, L, L), np.float32)
    for c in range(NCORES):
        attn_h[2 * c] = results[c]["A_out"][0].T
        attn_h[2 * c + 1] = results[c]["A_out"][1].T
    attn = np.broadcast_to(attn_h[:, None], (H, B, L, L)).reshape(H * B, L, L)
    return out, attn


def kernel(q=None, k=None, v=None, posi_att=None, w_k=None, b_k=None,
           w_v=None, b_v=None, w_fc=None, b_fc=None, ln_g=None, ln_b=None, **_):
    nc = _get_nc()
    in_maps = make_in_maps(v, posi_att, w_v, b_v, w_fc, b_fc, ln_g, ln_b)
    last_err = None
    for _attempt in range(4):
        try:
            res = run_bass_kernel_spmd(nc, in_maps, core_ids=list(range(NCORES)))
            return assemble_outputs(res.results)
        except Exception as e:  # transient backend execution errors
            last_err = e
            import time as _time
            _time.sleep(5.0)
    raise last_err
